# revision 27
# baseline (speedup 1.0000x reference)
"""Multi-head attention (B=2, S=2048, D=1024, H=16) on 8 NeuronCores.

Sharding: core c -> (batch b = c // 4, head-group g = c % 4). Each core
computes 4 heads of one batch plus the partial output projection for its
head-group's rows of Wo; the host sums the 4 partials per batch and adds bo.

Key-side compaction: masked key positions (True in `mask`) contribute
exactly zero attention weight, so the host drops them before sharding —
key/value inputs, K/V projections, score matmuls, the exp() pass and the
ctx matmuls all shrink by the masked fraction. The compacted length is
padded to a multiple of 128 with zero-columns whose mask bias (-60, applied
inside the exp activation) keeps their contribution at ~1e-26.

Layout strategy (per core):
  - Inputs are host-transposed: x^T [D, S*] so projections run with W as the
    stationary operand and x^T as the moving operand. Weights are
    host-prearranged into the [128, kt, n] SBUF layout so their DMA is one
    contiguous descriptor per partition row.
  - Q/K projections run in fp8-e4m3 with DoubleRow perf mode: x and W are
    quantized host-side (W scaled by 16 to stay in e4m3 normal range; the
    1/16 folds into the PSUM evacuation), and each matmul contracts TWO
    128-row D-tiles at once, halving the projection pass count. Errors of
    the fp8 quantization average out in the length-1024 dot products
    (~0.2% on q/k, ~0.6% on logits). Q^T/K^T themselves stay bf16.
  - Scores are computed TRANSPOSED: S^T[k, q] = K Q^T, so the key-position
    (padding) mask is per-PARTITION and folds into the single exp()
    activation as a bias AP, along with the 1/sqrt(dk) scale. One exp per
    [128, 2, w] PSUM tile covers both heads of a pair (the two heads' score
    matmuls run concurrently via PE row-tiling, K=64 each; the per-head row
    stride is padded to 512 so both destinations stay PSUM-bank-aligned).
  - V is produced in natural [S*, dv] layout with a ones-column per head
    (bias folded via an augmented contraction row), so the ctx matmul
    ctx^T = [V_h | 1]^T @ P^T also yields the softmax denominator as row 64.
  - Normalization: the denominator rows are copied to SBUF, reciprocal'd
    with the custom-DVE fast approximation (~18-bit, one instruction), cast
    to bf16, broadcast across partitions with two K=1 outer-product matmuls,
    then DVE multiplies. Pipelined one unit behind the matmul blocks.
  - Pipeline shape: K proj first, then 10 attention units (chunk, head-pair)
    with scores/exp running `lag` kt-slots ahead of ctx. ALL auxiliary PE
    work — V-proj m-tiles (unit 0), the previous unit's norm broadcast,
    O-proj m-tiles, Q-proj pair-tiles — is injected one item per kt slot
    INSIDE the attention stream, so the score matmuls (and the ACT exp
    stream they feed) never see a multi-us convoy of other PE work and the
    PE stays dense enough to hold the HAM clock gate at full rate.
  - Tail: the last q-chunk is split 384+128 and O-projection m-tiles are
    spread on an eligibility-driven schedule, so after the final unit's
    normalization only one 128-row m-tile remains.

Compute dtype (env KDT): "bf16" (default) uses bfloat16 matmul operands
(~5e-3 rel err, 1 cyc/row PE + half the DMA of f32); "f32r" keeps float32r
operands (~2e-4 rel err but ~2x slower matmuls). KF8=0 disables the fp8
projection path.
"""

import os
from contextlib import ExitStack

import numpy as np

import concourse.bacc as bacc
import concourse.mybir as mybir
import concourse.tile as tile

F32 = mybir.dt.float32
F32R = mybir.dt.float32r
BF16 = mybir.dt.bfloat16
FP8 = mybir.dt.float8e4
AF = mybir.ActivationFunctionType
ALU = mybir.AluOpType
MPD = mybir.MatmulPerfMode.DoubleRow

B, S, D = 2, 2048, 1024
H, DK = 16, 64
G = 4                    # head-groups (tensor parallel)
HPG = H // G             # 4 heads per group
DG = HPG * DK            # 256 head dims per group
NCORES = 8
MASK_NEG = -60.0         # additive post-scale bias for padded key positions
SCALE = 0.125            # 1/sqrt(dk)
WSC = 16.0               # host-side W_q/W_k scale for fp8 normal range

KT_D = D // 128          # 8 contraction tiles for projections
KP = KT_D // 2           # 4 DoubleRow kt-pairs
NT = DG // 128           # 2 partition-tiles of qT/kT/cT (one head-pair each)
QC = 512                 # q projection chunk (matmul moving dim)
NQC = S // QC            # 4
# attention-unit q chunks: last 512 chunk split 384+128 so the tail after
# the final normalization is a single O-proj m-tile
QCHUNKS = [(0, 512), (512, 512), (1024, 512), (1536, 384), (1920, 128)]
VW = HPG * (DK + 1)      # 260: V width incl. per-head ones column

KDT = os.environ.get("KDT", "bf16")
KF8 = os.environ.get("KF8", "1") == "1" and KDT == "bf16"


def _dt():
    return BF16 if KDT == "bf16" else F32R


def _np_dt():
    import ml_dtypes

    return ml_dtypes.bfloat16 if KDT == "bf16" else np.float32


def build_bass(ktk):
    """Build the SPMD program for `ktk` 128-wide key tiles (SK = 128*ktk)."""
    SK = 128 * ktk
    kchunks = [(n0, min(QC, SK - n0)) for n0 in range(0, SK, QC)]
    cdt = _dt()
    idt = FP8 if KF8 else cdt      # x_q/x_k + W_q/W_k dtype
    qsc = 1.0 / WSC if KF8 else 1.0

    nc = bacc.Bacc(None, target_bir_lowering=False, debug=False)

    xq = nc.dram_tensor("xq", [D, S], idt, kind="ExternalInput")
    xk = nc.dram_tensor("xk", [D, SK], idt, kind="ExternalInput")
    xv = nc.dram_tensor("xv", [D, SK], cdt, kind="ExternalInput")
    # weights pre-arranged host-side into the SBUF tile layout (contiguous
    # per-partition rows -> few large DMA descriptors)
    wq = nc.dram_tensor("wq", [128, KT_D, DG], idt, kind="ExternalInput")
    wk = nc.dram_tensor("wk", [128, KT_D, DG], idt, kind="ExternalInput")
    wv = nc.dram_tensor("wv", [128, KT_D, VW], cdt, kind="ExternalInput")
    wvb = nc.dram_tensor("wvb", [1, VW], cdt, kind="ExternalInput")
    wo = nc.dram_tensor("wo", [128, NT, D], cdt, kind="ExternalInput")
    bq = nc.dram_tensor("bq", [128, NT], F32, kind="ExternalInput")
    bk = nc.dram_tensor("bk", [128, NT], F32, kind="ExternalInput")
    mb = nc.dram_tensor("mb", [128, ktk], F32, kind="ExternalInput")
    cstc = nc.dram_tensor("cstc", [3, 128], cdt, kind="ExternalInput")
    # bf16 partials: the host sums 4 head-group partials per batch in f32,
    # so the extra rounding is ~0.1% while output DMA bytes halve
    odt = BF16 if KDT == "bf16" else F32
    out = nc.dram_tensor("out", [S, D], odt, kind="ExternalOutput")

    with tile.TileContext(nc) as tc, ExitStack() as ctx:
        consts = ctx.enter_context(tc.tile_pool(name="consts", bufs=1))
        resid = ctx.enter_context(tc.tile_pool(name="resid", bufs=1))
        stream = ctx.enter_context(tc.tile_pool(name="stream", bufs=4))
        vstream = ctx.enter_context(tc.tile_pool(name="vstream", bufs=8))
        ptp = ctx.enter_context(tc.tile_pool(name="ptp", bufs=10 if ktk <= 12 else 4))
        smalls = ctx.enter_context(tc.tile_pool(name="smalls", bufs=3 if ktk <= 12 else 2))
        obp = ctx.enter_context(tc.tile_pool(name="obp", bufs=3))

        # ---------------- constants / weights declarations ----------------
        wq_s = consts.tile([128, KT_D, DG], idt, tag="wq_s", name="wq_s")
        bq_s = consts.tile([128, NT], F32, tag="bq_s", name="bq_s")
        wk_s = consts.tile([128, KT_D, DG], idt, tag="wk_s", name="wk_s")
        bk_s = consts.tile([128, NT], F32, tag="bk_s", name="bk_s")
        wv_s = consts.tile([128, KT_D, VW], cdt, tag="wv_s", name="wv_s")
        wv_b = consts.tile([1, VW], cdt, tag="wv_b", name="wv_b")
        wo_s = consts.tile([128, NT, D], cdt, tag="wo_s", name="wo_s")
        mb_s = consts.tile([128, ktk], F32, tag="mb_s", name="mb_s")
        # Constant all-ones row comes from a tiny DRAM input — memset can't
        # write float32r tiles.
        ones1 = consts.tile([1, 128], cdt, tag="ones1", name="ones1")

        # ---------------- input stream prefetch ----------------
        # DMA issue order sets time-to-first-exp (the ACT exp stream is the
        # kernel's long pole): wk+xk first (K proj is the first PE work),
        # then wq + the q-chunk-0 slice of xq (first attention unit), xv
        # (V must finish right before the first ctx matmuls), then the rest
        # of xq streaming in under the attention phase. The x streams land
        # in kt-PAIR tiles [128, 2, w] so the fp8 DoubleRow matmuls can
        # address both contraction tiles in one AP.
        nc.sync.dma_start(out=wk_s, in_=wk[:])
        nc.sync.dma_start(out=bk_s, in_=bk[:])
        nc.sync.dma_start(out=mb_s, in_=mb[:])
        xk_t, xv_t, xq0_t, xqr_t = [], [], [], []
        for kp in range(KP):
            t_ = stream.tile([128, 2, SK], idt, tag="xk", name="xk_s")
            for j in range(2):
                nc.sync.dma_start(
                    out=t_[:, j, :],
                    in_=xk[(2 * kp + j) * 128 : (2 * kp + j + 1) * 128, :],
                )
            xk_t.append(t_)
        nc.sync.dma_start(out=wq_s, in_=wq[:])
        nc.sync.dma_start(out=bq_s, in_=bq[:])
        for kp in range(KP):
            t_ = stream.tile([128, 2, QC], idt, tag="xq0", name="xq0_s")
            for j in range(2):
                nc.sync.dma_start(
                    out=t_[:, j, :],
                    in_=xq[(2 * kp + j) * 128 : (2 * kp + j + 1) * 128, 0:QC],
                )
            xq0_t.append(t_)
        nc.sync.dma_start(out=wv_s, in_=wv[:])
        nc.sync.dma_start(out=wv_b, in_=wvb[:])
        nc.sync.dma_start(out=ones1, in_=cstc[0:1, :])
        for kt in range(KT_D):
            t_ = vstream.tile([128, SK], cdt, tag="xv", name="xv_s")
            nc.sync.dma_start(out=t_, in_=xv[kt * 128 : (kt + 1) * 128, :])
            xv_t.append(t_)
        for kp in range(KP):
            t_ = stream.tile([128, 2, S - QC], idt, tag="xqr", name="xqr_s")
            for j in range(2):
                nc.sync.dma_start(
                    out=t_[:, j, :],
                    in_=xq[(2 * kp + j) * 128 : (2 * kp + j + 1) * 128, QC:S],
                )
            xqr_t.append(t_)
        nc.sync.dma_start(out=wo_s, in_=wo[:])

        # ---------------- resident activations ----------------
        qT = [resid.tile([128, S], cdt, tag=f"qT{t}", name=f"qT{t}") for t in range(NT)]
        kT = [resid.tile([128, SK], cdt, tag=f"kT{t}", name=f"kT{t}") for t in range(NT)]
        v_s = resid.tile([128, ktk, VW], cdt, tag="v_s", name="v_s")
        cT = [resid.tile([128, S], cdt, tag=f"cT{t}", name=f"cT{t}") for t in range(NT)]

        def proj_mms(psum, w_s, t, src_of_kp, stop_w=None):
            # Q/K projection matmuls for one head-pair tile: fp8 DoubleRow
            # over kt-pairs (4 passes) or plain bf16 (8 passes)
            if KF8:
                for kp in range(KP):
                    nc.tensor.matmul(
                        psum,
                        lhsT=w_s[:, 2 * kp : 2 * kp + 2, t * 128 : (t + 1) * 128],
                        rhs=src_of_kp(kp),
                        start=(kp == 0),
                        stop=(kp == KP - 1),
                        perf_mode=MPD,
                    )
            else:
                for kp in range(KP):
                    for j in range(2):
                        nc.tensor.matmul(
                            psum,
                            lhsT=w_s[:, 2 * kp + j, t * 128 : (t + 1) * 128],
                            rhs=src_of_kp(kp)[:, j, :],
                            start=(kp == 0 and j == 0),
                            stop=(kp == KP - 1 and j == 1),
                        )

        # ---------------- phase 1: K^T projection ----------------
        with tc.tile_pool(name="pp", bufs=1, space="PSUM") as pp:
            psums = [
                pp.tile([128, QC], F32, tag=f"pp{i}", name=f"pp{i}")
                for i in range(NT * len(kchunks))
            ]
            for t in range(NT):
                for ci, (n0, w) in enumerate(kchunks):
                    proj_mms(
                        psums[t * len(kchunks) + ci][:, 0:w],
                        wk_s,
                        t,
                        lambda kp, n0=n0, w=w: xk_t[kp][:, :, n0 : n0 + w],
                    )
            for t in range(NT):
                for ci, (n0, w) in enumerate(kchunks):
                    nc.scalar.activation(
                        out=kT[t][:, n0 : n0 + w],
                        in_=psums[t * len(kchunks) + ci][:, 0:w],
                        func=AF.Identity,
                        bias=bk_s[:, t : t + 1],
                        scale=qsc,
                    )

        # ------- phases 2-4: V + Q projections woven into attention -------
        with tc.tile_pool(name="pa", bufs=1, space="PSUM") as pa:
            units = [(q0, w, p) for (q0, w) in QCHUNKS for p in range(NT)]

            def emit_vproj(m):
                # one V m-tile (all 4 heads + ones column); hooked into the
                # first unit's kt loop so it rides under the exp stream
                pvm = pa.tile([128, VW], F32, tag="pj", bufs=2, name="pv")
                for kt in range(KT_D):
                    nc.tensor.matmul(
                        pvm[:],
                        lhsT=xv_t[kt][:, m * 128 : (m + 1) * 128],
                        rhs=wv_s[:, kt, :],
                        start=(kt == 0),
                        stop=False,
                    )
                # bias + ones columns via augmented K=1 row
                nc.tensor.matmul(
                    pvm[:], lhsT=ones1[:], rhs=wv_b[:], start=False, stop=True
                )
                nc.vector.tensor_copy(v_s[:, m, :], pvm[:])

            def emit_qproj(qc, t):
                # one head-pair tile of one q-chunk of the Q projection: a
                # short psum-slot hold that slots between score matmuls
                # inside the attention stream
                qsl = slice(qc * QC, (qc + 1) * QC)
                qp = pa.tile([128, QC], F32, tag="pj", bufs=2, name="qp")

                def src(kp):
                    if qc == 0:
                        return xq0_t[kp][:, :, :]
                    sl = slice((qc - 1) * QC, qc * QC)
                    return xqr_t[kp][:, :, sl]

                proj_mms(qp[:], wq_s, t, src)
                # DVE evac (unscale + bias add + cast): ACT's exp backlog
                # would hold the qp psum slot hostage for several us
                if KF8:
                    nc.vector.tensor_scalar(
                        out=qT[t][:, qsl],
                        in0=qp[:],
                        scalar1=qsc,
                        scalar2=bq_s[:, t : t + 1],
                        op0=ALU.mult,
                        op1=ALU.add,
                    )
                else:
                    nc.vector.tensor_scalar_add(
                        qT[t][:, qsl], qp[:], bq_s[:, t : t + 1]
                    )

            def emit_attn(q0, w, p, hook=None, lag=1):
                qsl = slice(q0, q0 + w)
                hA, hB = 2 * p, 2 * p + 1
                # bufs=1: the normalization completes inside the same
                # emission iteration (gpsimd broadcast + DVE muls), so the
                # next unit's accumulation reuses the banks without a long
                # cross-unit hold
                pcA = pa.tile([65, QC], F32, tag="pcA", bufs=1, name="pcA")
                pcB = pa.tile([65, QC], F32, tag="pcB", bufs=1, name="pcB")
                pts = []
                # software pipeline: scores/exp `lag` kts ahead of ctx
                LAG = lag
                for kt in range(ktk + LAG):
                    if kt < ktk:
                        ksl = slice(kt * 128, (kt + 1) * 128)
                        # per-head row stride padded to QC so both heads'
                        # matmul destinations stay PSUM-bank-aligned even
                        # for the narrow tail units
                        ps = pa.tile([128, 2, QC], F32, tag="ps", bufs=2, name="ps")
                        nc.tensor.matmul(
                            ps[:, 0, 0:w],
                            lhsT=kT[p][0:64, ksl],
                            rhs=qT[p][0:64, qsl],
                            start=True,
                            stop=True,
                        )
                        nc.tensor.matmul(
                            ps[:, 1, 0:w],
                            lhsT=kT[p][64:128, ksl],
                            rhs=qT[p][64:128, qsl],
                            start=True,
                            stop=True,
                        )
                        pt = ptp.tile([128, 2, w], cdt, tag="pt", name="pt")
                        nc.scalar.activation(
                            out=pt[:],
                            in_=ps[:, :, 0:w],
                            func=AF.Exp,
                            bias=mb_s[:, kt : kt + 1],
                            scale=SCALE,
                        )
                        pts.append(pt)
                    if hook is not None and kt < ktk + LAG - 1:
                        hook(kt)
                    if kt >= LAG:
                        ct = kt - LAG
                        ptc = pts[ct]
                        nc.tensor.matmul(
                            pcA[0:65, 0:w],
                            lhsT=v_s[:, ct, hA * 65 : (hA + 1) * 65],
                            rhs=ptc[:, 0, :],
                            start=(ct == 0),
                            stop=(ct == ktk - 1),
                        )
                        nc.tensor.matmul(
                            pcB[0:65, 0:w],
                            lhsT=v_s[:, ct, hB * 65 : (hB + 1) * 65],
                            rhs=ptc[:, 1, :],
                            start=(ct == 0),
                            stop=(ct == ktk - 1),
                        )
                return pcA, pcB

            def emit_norm(q0, w, p, pcA, pcB):
                # off-PE normalization: DVE copies the two denominator rows
                # to SBUF and runs the fast-approx reciprocal (custom DVE,
                # ~18 bits; must read SBUF — PSUM-source custom-DVE
                # misbehaves on hw); GPSIMD (otherwise idle) broadcasts the
                # reciprocal row across partitions; DVE multiplies.
                qsl = slice(q0, q0 + w)
                den = smalls.tile([1, 2 * QC], F32, tag="den", name="den")
                rec32 = smalls.tile([1, 2 * QC], F32, tag="rec32", name="rec32")
                bcs = smalls.tile([128, 2 * QC], F32, tag="bcs", name="bcs")
                nc.vector.tensor_copy(den[0:1, 0:w], pcA[64:65, 0:w])
                nc.vector.tensor_copy(den[0:1, w : 2 * w], pcB[64:65, 0:w])
                nc.vector.reciprocal_approx_fast(
                    out=rec32[0:1, 0 : 2 * w], in_=den[0:1, 0 : 2 * w]
                )
                nc.gpsimd.partition_broadcast(
                    bcs[:, 0 : 2 * w], rec32[0:1, 0 : 2 * w]
                )
                nc.vector.tensor_mul(
                    cT[p][0:64, qsl], pcA[0:64, 0:w], bcs[0:64, 0:w]
                )
                nc.vector.tensor_mul(
                    cT[p][64:128, qsl], pcB[0:64, 0:w], bcs[64:128, w : 2 * w]
                )

            def emit_final(ms):
                for m in ms:
                    ob = obp.tile([128, D], odt, tag="ob", name="ob")
                    # two single-bank column halves through the "pj" slot
                    # pair, so O-proj never touches the score psum rotation
                    for oc in range(2):
                        pom = pa.tile([128, 512], F32, tag="pj", bufs=2, name="pom")
                        for t in range(NT):
                            nc.tensor.matmul(
                                pom[:],
                                lhsT=cT[t][:, m * 128 : (m + 1) * 128],
                                rhs=wo_s[:, t, oc * 512 : (oc + 1) * 512],
                                start=(t == 0),
                                stop=(t == NT - 1),
                            )
                        # DVE copy (gpsimd can't read PSUM): ACT is
                        # saturated by the exp() stream
                        nc.vector.tensor_copy(
                            ob[:, oc * 512 : (oc + 1) * 512], pom[:]
                        )
                    nc.sync.dma_start(out=out[m * 128 : (m + 1) * 128, :], in_=ob[:])

            # O-projection m-tile schedule: eligibility-driven spread;
            # normalization now completes at the end of its own iteration,
            # so m-tiles of q-chunk c are ready from iteration 2c+2,
            # leaving only m15 after the loop.
            OSCHED = {2: [0, 1], 3: [2, 3], 4: [4, 5], 5: [6, 7],
                      6: [8, 9], 7: [10, 11], 8: [12, 13], 9: [14]}

            # Unit 0 runs with a deep ctx lag: its exp stream starts as soon
            # as xq chunk 0 + kT are ready (~8us before xv finishes), and
            # the V projection tiles are hooked in just after xv lands, each
            # one kt slot ahead of the ctx matmul that consumes it.
            LAG0 = min(4, ktk)

            def vhook(kt):
                if LAG0 - 1 <= kt < LAG0 - 1 + ktk:
                    emit_vproj(kt - (LAG0 - 1))

            emit_qproj(0, 0)
            emit_qproj(0, 1)
            for i, (q0, w, p) in enumerate(units):
                # Auxiliary PE work (O-proj m-tiles, next q-chunk projection
                # halves) is injected one item per kt slot INSIDE the
                # attention stream, on its own psum tag, so the score
                # matmuls — and with them the ACT exp stream — never see a
                # multi-us convoy of other PE work nor lose their psum
                # double-buffering, and the PE activity stays dense enough
                # to hold the HAM clock gate at full rate.
                items = []
                for m in OSCHED.get(i, []):
                    items.append(lambda m=m: emit_final([m]))
                if i in (1, 3, 5):
                    qc = i // 2 + 1
                    items.append(lambda qc=qc: emit_qproj(qc, 0))
                    items.append(lambda qc=qc: emit_qproj(qc, 1))
                work = list(reversed(items))

                if i == 0:
                    hook = vhook
                else:
                    def hook(kt, work=work):
                        # start at slot 2 so the scores get a head start
                        if kt >= 2 and work:
                            work.pop()()

                pcA, pcB = emit_attn(q0, w, p, hook, lag=LAG0 if i == 0 else 2)
                while i > 0 and work:
                    work.pop()()
                # normalization immediately after this unit's ctx stop: the
                # whole chain runs on DVE+GPSIMD, off the PE stream
                emit_norm(q0, w, p, pcA, pcB)
            emit_final([15])

    nc.compile()
    return nc


def _const_rows():
    cst = np.zeros((3, 128), np.float32)
    cst[0, :] = 1.0
    cst[1, 0:64] = 1.0
    cst[2, 64:128] = 1.0
    return cst


def make_in_maps(query, key, value, mask, Wq, bq, Wk, bk, Wv, bv, Wo, bo):
    """Returns (in_maps, ktk). Key positions with mask=True are dropped."""
    query = np.asarray(query, np.float32)
    key = np.asarray(key, np.float32)
    value = np.asarray(value, np.float32)
    mask = np.asarray(mask)
    Wq = np.asarray(Wq, np.float32)
    Wk = np.asarray(Wk, np.float32)
    Wv = np.asarray(Wv, np.float32)
    Wo = np.asarray(Wo, np.float32)
    bq = np.asarray(bq, np.float32)
    bk = np.asarray(bk, np.float32)
    bv = np.asarray(bv, np.float32)

    keep = [np.flatnonzero(~mask[b, 0]) for b in range(B)]
    ktk = max(1, max((len(k) + 127) // 128 for k in keep))
    SKc = 128 * ktk
    ndt = _np_dt()
    if KF8:
        idt = mybir.dt.np(FP8)

        def _q8(a):
            return np.clip(a, -240.0, 240.0).astype(idt)
    else:
        idt = ndt
        _q8 = None

    def _prearrange(w):
        # [D, n] -> [128, KT_D, n] matching the SBUF tile layout
        n = w.shape[1]
        return np.ascontiguousarray(
            w.reshape(KT_D, 128, n).transpose(1, 0, 2)
        )

    in_maps = []
    for c in range(NCORES):
        b, g = c // G, c % G
        cs = slice(g * DG, (g + 1) * DG)
        idx = keep[b]
        nk = len(idx)
        xkc = np.zeros((D, SKc), np.float32)
        xvc = np.zeros((D, SKc), np.float32)
        xkc[:, :nk] = key[b].T[:, idx]
        xvc[:, :nk] = value[b].T[:, idx]
        mbias = np.full(SKc, MASK_NEG, np.float32)
        mbias[:nk] = 0.0

        wv_aug = np.zeros((D, VW), np.float32)
        wvb_row = np.zeros((1, VW), np.float32)
        for j in range(HPG):
            src = slice(g * DG + j * DK, g * DG + (j + 1) * DK)
            wv_aug[:, j * 65 : j * 65 + 64] = Wv[:, src]
            wvb_row[0, j * 65 : j * 65 + 64] = bv[src]
            wvb_row[0, j * 65 + 64] = 1.0

        # wo: [DG, D] -> [128, NT, D]
        wo_pre = np.ascontiguousarray(
            Wo[cs, :].reshape(NT, 128, D).transpose(1, 0, 2)
        )

        xq_c = np.ascontiguousarray(query[b].T)
        wq_c = _prearrange(Wq[:, cs])
        wk_c = _prearrange(Wk[:, cs])
        if KF8:
            xq_m, xk_m = _q8(xq_c), _q8(xkc)
            wq_m, wk_m = _q8(wq_c * WSC), _q8(wk_c * WSC)
        else:
            xq_m, xk_m = xq_c.astype(ndt), xkc.astype(ndt)
            wq_m, wk_m = wq_c.astype(ndt), wk_c.astype(ndt)

        in_maps.append(
            {
                "xq": xq_m,
                "xk": xk_m,
                "xv": xvc.astype(ndt),
                "wq": wq_m,
                "wk": wk_m,
                "wv": _prearrange(wv_aug).astype(ndt),
                "wvb": wvb_row.astype(ndt),
                "wo": wo_pre.astype(ndt),
                "bq": np.ascontiguousarray(bq[cs].reshape(NT, 128).T),
                "bk": np.ascontiguousarray(bk[cs].reshape(NT, 128).T),
                "mb": np.ascontiguousarray(mbias.reshape(ktk, 128).T),
                "cstc": _const_rows().astype(ndt),
            }
        )
    return in_maps, ktk


def combine_outputs(results, mask, bo):
    mask = np.asarray(mask)
    bo = np.asarray(bo, np.float32)
    out = np.zeros((B, S, D), np.float32)
    for c in range(NCORES):
        out[c // G] += np.asarray(results[c]["out"], np.float32)
    for b in range(B):
        if mask[b, 0].all():
            # reference: fully-masked rows produce zero context
            out[b] = 0.0
    out += bo[None, None, :]
    return out


_NC_CACHE = {}


def kernel(query, key, value, mask, Wq, bq, Wk, bk, Wv, bv, Wo, bo):
    from concourse.bass_utils import run_bass_kernel_spmd

    in_maps, ktk = make_in_maps(
        query, key, value, mask, Wq, bq, Wk, bk, Wv, bv, Wo, bo
    )
    nc = _NC_CACHE.get((KDT, KF8, ktk))
    if nc is None:
        nc = _NC_CACHE[(KDT, KF8, ktk)] = build_bass(ktk)
    res = run_bass_kernel_spmd(nc, in_maps, list(range(NCORES))).results
    return combine_outputs(res, mask, bo)


# revision 30
# speedup vs baseline: 1.0373x; 1.0373x over previous
"""Multi-head attention (B=2, S=2048, D=1024, H=16) on 8 NeuronCores.

Sharding: core c -> (batch b = c // 4, head-group g = c % 4). Each core
computes 4 heads of one batch plus the partial output projection for its
head-group's rows of Wo; the host sums the 4 partials per batch and adds bo.

Key-side compaction: masked key positions (True in `mask`) contribute
exactly zero attention weight, so the host drops them before sharding —
key/value inputs, K/V projections, score matmuls, the exp() pass and the
ctx matmuls all shrink by the masked fraction. The compacted length is
padded to a multiple of 128 with zero-columns whose mask bias (-60, applied
inside the exp activation) keeps their contribution at ~1e-26.

Layout strategy (per core):
  - Inputs are host-transposed: x^T [D, S*] so projections run with W as the
    stationary operand and x^T as the moving operand. Weights are
    host-prearranged into the [128, kt, n] SBUF layout so their DMA is one
    contiguous descriptor per partition row.
  - Q/K projections run in fp8-e4m3 with DoubleRow perf mode: x and W are
    quantized host-side (W scaled by 16 to stay in e4m3 normal range; the
    1/16 folds into the PSUM evacuation), and each matmul contracts TWO
    128-row D-tiles at once, halving the projection pass count. Errors of
    the fp8 quantization average out in the length-1024 dot products
    (~0.2% on q/k, ~0.6% on logits). Q^T/K^T themselves stay bf16.
  - Scores are computed TRANSPOSED: S^T[k, q] = K Q^T, so the key-position
    (padding) mask is per-PARTITION and folds into the single exp()
    activation as a bias AP, along with the 1/sqrt(dk) scale. One exp per
    [128, 2, w] PSUM tile covers both heads of a pair (the two heads' score
    matmuls run concurrently via PE row-tiling, K=64 each; the per-head row
    stride is padded to 512 so both destinations stay PSUM-bank-aligned).
  - V is produced in natural [S*, dv] layout with a ones-column per head
    (bias folded via an augmented contraction row), so the ctx matmul
    ctx^T = [V_h | 1]^T @ P^T also yields the softmax denominator as row 64.
  - Normalization: the denominator rows are copied to SBUF, reciprocal'd
    with the custom-DVE fast approximation (~18-bit, one instruction), cast
    to bf16, broadcast across partitions with two K=1 outer-product matmuls,
    then DVE multiplies. Pipelined one unit behind the matmul blocks.
  - Pipeline shape: K proj first, then 10 attention units (chunk, head-pair)
    with scores/exp running `lag` kt-slots ahead of ctx. ALL auxiliary PE
    work — V-proj m-tiles (unit 0), the previous unit's norm broadcast,
    O-proj m-tiles, Q-proj pair-tiles — is injected one item per kt slot
    INSIDE the attention stream, so the score matmuls (and the ACT exp
    stream they feed) never see a multi-us convoy of other PE work and the
    PE stays dense enough to hold the HAM clock gate at full rate.
  - Tail: the last q-chunk is split 384+128 and O-projection m-tiles are
    spread on an eligibility-driven schedule, so after the final unit's
    normalization only one 128-row m-tile remains.

Compute dtype (env KDT): "bf16" (default) uses bfloat16 matmul operands
(~5e-3 rel err, 1 cyc/row PE + half the DMA of f32); "f32r" keeps float32r
operands (~2e-4 rel err but ~2x slower matmuls). KF8=0 disables the fp8
projection path.
"""

import os
from contextlib import ExitStack

import numpy as np

import concourse.bacc as bacc
import concourse.mybir as mybir
import concourse.tile as tile

F32 = mybir.dt.float32
F32R = mybir.dt.float32r
BF16 = mybir.dt.bfloat16
FP8 = mybir.dt.float8e4
AF = mybir.ActivationFunctionType
ALU = mybir.AluOpType
MPD = mybir.MatmulPerfMode.DoubleRow

B, S, D = 2, 2048, 1024
H, DK = 16, 64
G = 4                    # head-groups (tensor parallel)
HPG = H // G             # 4 heads per group
DG = HPG * DK            # 256 head dims per group
NCORES = 8
MASK_NEG = -60.0         # additive post-scale bias for padded key positions
SCALE = 0.125            # 1/sqrt(dk)
WSC = 16.0               # host-side W_q/W_k scale for fp8 normal range

KT_D = D // 128          # 8 contraction tiles for projections
KP = KT_D // 2           # 4 DoubleRow kt-pairs
NT = DG // 128           # 2 partition-tiles of qT/kT/cT (one head-pair each)
QC = 512                 # q projection chunk (matmul moving dim)
NQC = S // QC            # 4
# attention-unit q chunks: last 512 chunk split 384+128 so the tail after
# the final normalization is a single O-proj m-tile
QCHUNKS = [(0, 512), (512, 512), (1024, 512), (1536, 384), (1920, 128)]
VW = HPG * (DK + 1)      # 260: V width incl. per-head ones column

KDT = os.environ.get("KDT", "bf16")
KF8 = os.environ.get("KF8", "1") == "1" and KDT == "bf16"


def _dt():
    return BF16 if KDT == "bf16" else F32R


def _np_dt():
    import ml_dtypes

    return ml_dtypes.bfloat16 if KDT == "bf16" else np.float32


def build_bass(ktk):
    """Build the SPMD program for `ktk` 128-wide key tiles (SK = 128*ktk)."""
    SK = 128 * ktk
    kchunks = [(n0, min(QC, SK - n0)) for n0 in range(0, SK, QC)]
    cdt = _dt()
    idt = FP8 if KF8 else cdt      # x_q/x_k + W_q/W_k dtype
    qsc = 1.0 / WSC if KF8 else 1.0

    nc = bacc.Bacc(None, target_bir_lowering=False, debug=False)

    xq = nc.dram_tensor("xq", [D, S], idt, kind="ExternalInput")
    xk = nc.dram_tensor("xk", [D, SK], idt, kind="ExternalInput")
    xv = nc.dram_tensor("xv", [D, SK], cdt, kind="ExternalInput")
    # weights pre-arranged host-side into the SBUF tile layout (contiguous
    # per-partition rows -> few large DMA descriptors)
    wq = nc.dram_tensor("wq", [128, KT_D, DG], idt, kind="ExternalInput")
    wk = nc.dram_tensor("wk", [128, KT_D, DG], idt, kind="ExternalInput")
    wv = nc.dram_tensor("wv", [128, KT_D, VW], cdt, kind="ExternalInput")
    wvb = nc.dram_tensor("wvb", [1, VW], cdt, kind="ExternalInput")
    wo = nc.dram_tensor("wo", [128, NT, D], cdt, kind="ExternalInput")
    bq = nc.dram_tensor("bq", [128, NT], F32, kind="ExternalInput")
    bk = nc.dram_tensor("bk", [128, NT], F32, kind="ExternalInput")
    mb = nc.dram_tensor("mb", [128, ktk], F32, kind="ExternalInput")
    cstc = nc.dram_tensor("cstc", [3, 128], cdt, kind="ExternalInput")
    # bf16 partials: the host sums 4 head-group partials per batch in f32,
    # so the extra rounding is ~0.1% while output DMA bytes halve
    odt = BF16 if KDT == "bf16" else F32
    out = nc.dram_tensor("out", [S, D], odt, kind="ExternalOutput")

    with tile.TileContext(nc) as tc, ExitStack() as ctx:
        consts = ctx.enter_context(tc.tile_pool(name="consts", bufs=1))
        resid = ctx.enter_context(tc.tile_pool(name="resid", bufs=1))
        stream = ctx.enter_context(tc.tile_pool(name="stream", bufs=4))
        vstream = ctx.enter_context(tc.tile_pool(name="vstream", bufs=8))
        ptp = ctx.enter_context(tc.tile_pool(name="ptp", bufs=10 if ktk <= 12 else 4))
        smalls = ctx.enter_context(tc.tile_pool(name="smalls", bufs=3 if ktk <= 12 else 2))
        obp = ctx.enter_context(tc.tile_pool(name="obp", bufs=3))

        # ---------------- constants / weights declarations ----------------
        wq_s = consts.tile([128, KT_D, DG], idt, tag="wq_s", name="wq_s")
        bq_s = consts.tile([128, NT], F32, tag="bq_s", name="bq_s")
        wk_s = consts.tile([128, KT_D, DG], idt, tag="wk_s", name="wk_s")
        bk_s = consts.tile([128, NT], F32, tag="bk_s", name="bk_s")
        wv_s = consts.tile([128, KT_D, VW], cdt, tag="wv_s", name="wv_s")
        wv_b = consts.tile([1, VW], cdt, tag="wv_b", name="wv_b")
        wo_s = consts.tile([128, NT, D], cdt, tag="wo_s", name="wo_s")
        mb_s = consts.tile([128, ktk], F32, tag="mb_s", name="mb_s")
        # Constant all-ones row comes from a tiny DRAM input — memset can't
        # write float32r tiles.
        ones1 = consts.tile([1, 128], cdt, tag="ones1", name="ones1")

        # ---------------- input stream prefetch ----------------
        # DMA issue order sets time-to-first-exp (the ACT exp stream is the
        # kernel's long pole): wk+xk first (K proj is the first PE work),
        # then wq + the q-chunk-0 slice of xq (first attention unit), xv
        # (V must finish right before the first ctx matmuls), then the rest
        # of xq streaming in under the attention phase. The x streams land
        # in kt-PAIR tiles [128, 2, w] so the fp8 DoubleRow matmuls can
        # address both contraction tiles in one AP.
        nc.sync.dma_start(out=wk_s, in_=wk[:])
        nc.sync.dma_start(out=bk_s, in_=bk[:])
        nc.sync.dma_start(out=mb_s, in_=mb[:])
        xk_t, xv_t, xq0_t, xqr_t = [], [], [], []
        for kp in range(KP):
            t_ = stream.tile([128, 2, SK], idt, tag="xk", name="xk_s")
            for j in range(2):
                nc.sync.dma_start(
                    out=t_[:, j, :],
                    in_=xk[(2 * kp + j) * 128 : (2 * kp + j + 1) * 128, :],
                )
            xk_t.append(t_)
        nc.sync.dma_start(out=wq_s, in_=wq[:])
        nc.sync.dma_start(out=bq_s, in_=bq[:])
        for kp in range(KP):
            t_ = stream.tile([128, 2, QC], idt, tag="xq0", name="xq0_s")
            for j in range(2):
                nc.sync.dma_start(
                    out=t_[:, j, :],
                    in_=xq[(2 * kp + j) * 128 : (2 * kp + j + 1) * 128, 0:QC],
                )
            xq0_t.append(t_)
        nc.sync.dma_start(out=wv_s, in_=wv[:])
        nc.sync.dma_start(out=wv_b, in_=wvb[:])
        nc.sync.dma_start(out=ones1, in_=cstc[0:1, :])
        for kt in range(KT_D):
            t_ = vstream.tile([128, SK], cdt, tag="xv", name="xv_s")
            nc.sync.dma_start(out=t_, in_=xv[kt * 128 : (kt + 1) * 128, :])
            xv_t.append(t_)
        for kp in range(KP):
            t_ = stream.tile([128, 2, S - QC], idt, tag="xqr", name="xqr_s")
            for j in range(2):
                nc.sync.dma_start(
                    out=t_[:, j, :],
                    in_=xq[(2 * kp + j) * 128 : (2 * kp + j + 1) * 128, QC:S],
                )
            xqr_t.append(t_)
        nc.sync.dma_start(out=wo_s, in_=wo[:])

        # ---------------- resident activations ----------------
        qT = [resid.tile([128, S], cdt, tag=f"qT{t}", name=f"qT{t}") for t in range(NT)]
        kT = [resid.tile([128, SK], cdt, tag=f"kT{t}", name=f"kT{t}") for t in range(NT)]
        v_s = resid.tile([128, ktk, VW], cdt, tag="v_s", name="v_s")
        cT = [resid.tile([128, S], cdt, tag=f"cT{t}", name=f"cT{t}") for t in range(NT)]

        def proj_mms(psum, w_s, t, src_of_kp, stop_w=None):
            # Q/K projection matmuls for one head-pair tile: fp8 DoubleRow
            # over kt-pairs (4 passes) or plain bf16 (8 passes)
            if KF8:
                for kp in range(KP):
                    nc.tensor.matmul(
                        psum,
                        lhsT=w_s[:, 2 * kp : 2 * kp + 2, t * 128 : (t + 1) * 128],
                        rhs=src_of_kp(kp),
                        start=(kp == 0),
                        stop=(kp == KP - 1),
                        perf_mode=MPD,
                    )
            else:
                for kp in range(KP):
                    for j in range(2):
                        nc.tensor.matmul(
                            psum,
                            lhsT=w_s[:, 2 * kp + j, t * 128 : (t + 1) * 128],
                            rhs=src_of_kp(kp)[:, j, :],
                            start=(kp == 0 and j == 0),
                            stop=(kp == KP - 1 and j == 1),
                        )

        # ---------------- phase 1: K^T projection ----------------
        with tc.tile_pool(name="pp", bufs=1, space="PSUM") as pp:
            psums = [
                pp.tile([128, QC], F32, tag=f"pp{i}", name=f"pp{i}")
                for i in range(NT * len(kchunks))
            ]
            for t in range(NT):
                for ci, (n0, w) in enumerate(kchunks):
                    proj_mms(
                        psums[t * len(kchunks) + ci][:, 0:w],
                        wk_s,
                        t,
                        lambda kp, n0=n0, w=w: xk_t[kp][:, :, n0 : n0 + w],
                    )
            for t in range(NT):
                for ci, (n0, w) in enumerate(kchunks):
                    nc.scalar.activation(
                        out=kT[t][:, n0 : n0 + w],
                        in_=psums[t * len(kchunks) + ci][:, 0:w],
                        func=AF.Identity,
                        bias=bk_s[:, t : t + 1],
                        scale=qsc,
                    )

        # ------- phases 2-4: V + Q projections woven into attention -------
        with tc.tile_pool(name="pa", bufs=1, space="PSUM") as pa:
            units = [(q0, w, p) for (q0, w) in QCHUNKS for p in range(NT)]

            def emit_vproj(m):
                # one V m-tile (all 4 heads + ones column); hooked into the
                # first unit's kt loop so it rides under the exp stream
                pvm = pa.tile([128, VW], F32, tag="pj", bufs=2, name="pv")
                for kt in range(KT_D):
                    nc.tensor.matmul(
                        pvm[:],
                        lhsT=xv_t[kt][:, m * 128 : (m + 1) * 128],
                        rhs=wv_s[:, kt, :],
                        start=(kt == 0),
                        stop=False,
                    )
                # bias + ones columns via augmented K=1 row
                nc.tensor.matmul(
                    pvm[:], lhsT=ones1[:], rhs=wv_b[:], start=False, stop=True
                )
                nc.vector.tensor_copy(v_s[:, m, :], pvm[:])

            def emit_qproj(qc, t):
                # one head-pair tile of one q-chunk of the Q projection: a
                # short psum-slot hold that slots between score matmuls
                # inside the attention stream
                qsl = slice(qc * QC, (qc + 1) * QC)
                qp = pa.tile([128, QC], F32, tag="pj", bufs=2, name="qp")

                def src(kp):
                    if qc == 0:
                        return xq0_t[kp][:, :, :]
                    sl = slice((qc - 1) * QC, qc * QC)
                    return xqr_t[kp][:, :, sl]

                proj_mms(qp[:], wq_s, t, src)
                # DVE evac (unscale + bias add + cast): ACT's exp backlog
                # would hold the qp psum slot hostage for several us
                if KF8:
                    nc.vector.tensor_scalar(
                        out=qT[t][:, qsl],
                        in0=qp[:],
                        scalar1=qsc,
                        scalar2=bq_s[:, t : t + 1],
                        op0=ALU.mult,
                        op1=ALU.add,
                    )
                else:
                    nc.vector.tensor_scalar_add(
                        qT[t][:, qsl], qp[:], bq_s[:, t : t + 1]
                    )

            def emit_attn(q0, w, p, hook=None, lag=1):
                qsl = slice(q0, q0 + w)
                hA, hB = 2 * p, 2 * p + 1
                # bufs=1: the normalization completes inside the same
                # emission iteration (gpsimd broadcast + DVE muls), so the
                # next unit's accumulation reuses the banks without a long
                # cross-unit hold
                pcA = pa.tile([65, QC], F32, tag="pcA", bufs=1, name="pcA")
                pcB = pa.tile([65, QC], F32, tag="pcB", bufs=1, name="pcB")
                pts = []
                # software pipeline: scores/exp `lag` kts ahead of ctx
                LAG = lag
                for kt in range(ktk + LAG):
                    if kt < ktk:
                        ksl = slice(kt * 128, (kt + 1) * 128)
                        # per-head row stride padded to QC so both heads'
                        # matmul destinations stay PSUM-bank-aligned even
                        # for the narrow tail units
                        ps = pa.tile([128, 2, QC], F32, tag="ps", bufs=2, name="ps")
                        nc.tensor.matmul(
                            ps[:, 0, 0:w],
                            lhsT=kT[p][0:64, ksl],
                            rhs=qT[p][0:64, qsl],
                            start=True,
                            stop=True,
                        )
                        nc.tensor.matmul(
                            ps[:, 1, 0:w],
                            lhsT=kT[p][64:128, ksl],
                            rhs=qT[p][64:128, qsl],
                            start=True,
                            stop=True,
                        )
                        pt = ptp.tile([128, 2, w], cdt, tag="pt", name="pt")
                        nc.scalar.activation(
                            out=pt[:],
                            in_=ps[:, :, 0:w],
                            func=AF.Exp,
                            bias=mb_s[:, kt : kt + 1],
                            scale=SCALE,
                        )
                        pts.append(pt)
                    if hook is not None and kt < ktk + LAG - 1:
                        hook(kt)
                    if kt >= LAG:
                        ct = kt - LAG
                        ptc = pts[ct]
                        nc.tensor.matmul(
                            pcA[0:65, 0:w],
                            lhsT=v_s[:, ct, hA * 65 : (hA + 1) * 65],
                            rhs=ptc[:, 0, :],
                            start=(ct == 0),
                            stop=(ct == ktk - 1),
                        )
                        nc.tensor.matmul(
                            pcB[0:65, 0:w],
                            lhsT=v_s[:, ct, hB * 65 : (hB + 1) * 65],
                            rhs=ptc[:, 1, :],
                            start=(ct == 0),
                            stop=(ct == ktk - 1),
                        )
                return pcA, pcB

            def emit_norm_a(q0, w, p, pcA, pcB):
                # Normalization part A, at the end of the unit's own
                # iteration: DVE copies the denominator rows to SBUF, runs
                # the fast-approx reciprocal (custom DVE, ~18 bits; must
                # read SBUF — PSUM-source custom-DVE misbehaves on hw) and
                # evacuates the UNNORMALIZED ctx to cT, releasing the
                # single-buffered pcA/pcB banks immediately. GPSIMD
                # (otherwise idle) casts the reciprocal row to bf16 and
                # broadcasts it across partitions, off every critical
                # engine.
                qsl = slice(q0, q0 + w)
                den = smalls.tile([1, 2 * QC], F32, tag="den", name="den")
                rec32 = smalls.tile([1, 2 * QC], F32, tag="rec32", name="rec32")
                rec = smalls.tile([1, 2 * QC], cdt, tag="rec", name="rec")
                bcs = smalls.tile([128, 2 * QC], cdt, tag="bcs", name="bcs")
                nc.vector.tensor_copy(den[0:1, 0:w], pcA[64:65, 0:w])
                nc.vector.tensor_copy(den[0:1, w : 2 * w], pcB[64:65, 0:w])
                nc.vector.reciprocal_approx_fast(
                    out=rec32[0:1, 0 : 2 * w], in_=den[0:1, 0 : 2 * w]
                )
                nc.vector.tensor_copy(cT[p][0:64, qsl], pcA[0:64, 0:w])
                nc.vector.tensor_copy(cT[p][64:128, qsl], pcB[0:64, 0:w])
                nc.gpsimd.tensor_copy(rec[0:1, 0 : 2 * w], rec32[0:1, 0 : 2 * w])
                nc.gpsimd.partition_broadcast(
                    bcs[:, 0 : 2 * w], rec[0:1, 0 : 2 * w]
                )
                return bcs

            def emit_norm_b(q0, w, p, bcs):
                # Normalization part B, injected one iteration later (by
                # which time the gpsimd broadcast is long done): in-place
                # bf16 SBUF multiplies at the DVE's fast mode.
                qsl = slice(q0, q0 + w)
                nc.vector.tensor_mul(
                    cT[p][0:64, qsl], cT[p][0:64, qsl], bcs[0:64, 0:w]
                )
                nc.vector.tensor_mul(
                    cT[p][64:128, qsl], cT[p][64:128, qsl], bcs[64:128, w : 2 * w]
                )

            def emit_final(ms):
                for m in ms:
                    ob = obp.tile([128, D], odt, tag="ob", name="ob")
                    # two single-bank column halves through the "pj" slot
                    # pair, so O-proj never touches the score psum rotation
                    for oc in range(2):
                        pom = pa.tile([128, 512], F32, tag="pj", bufs=2, name="pom")
                        for t in range(NT):
                            nc.tensor.matmul(
                                pom[:],
                                lhsT=cT[t][:, m * 128 : (m + 1) * 128],
                                rhs=wo_s[:, t, oc * 512 : (oc + 1) * 512],
                                start=(t == 0),
                                stop=(t == NT - 1),
                            )
                        # DVE copy (gpsimd can't read PSUM): ACT is
                        # saturated by the exp() stream
                        nc.vector.tensor_copy(
                            ob[:, oc * 512 : (oc + 1) * 512], pom[:]
                        )
                    nc.sync.dma_start(out=out[m * 128 : (m + 1) * 128, :], in_=ob[:])

            # O-projection m-tile schedule: eligibility-driven spread; the
            # in-place normalization multiply of unit j is injected at
            # iteration j+1, so m-tiles of q-chunk c are ready from
            # iteration 2c+2 (after the norm-b item of unit 2c+1, which is
            # queued first), leaving only m15 after the loop.
            OSCHED = {2: [0, 1], 3: [2, 3], 4: [4, 5], 5: [6, 7],
                      6: [8, 9], 7: [10, 11], 8: [12, 13], 9: [14]}

            # Unit 0 runs with a deep ctx lag: its exp stream starts as soon
            # as xq chunk 0 + kT are ready (~8us before xv finishes), and
            # the V projection tiles are hooked in just after xv lands, each
            # one kt slot ahead of the ctx matmul that consumes it.
            LAG0 = min(4, ktk)

            def vhook(kt):
                if LAG0 - 1 <= kt < LAG0 - 1 + ktk:
                    emit_vproj(kt - (LAG0 - 1))

            emit_qproj(0, 0)
            emit_qproj(0, 1)
            pend = {}
            for i, (q0, w, p) in enumerate(units):
                # Auxiliary work (the previous unit's in-place norm multiply
                # on DVE, O-proj m-tiles, next q-chunk projection halves) is
                # injected one item per kt slot INSIDE the attention stream,
                # on its own psum tag, so the score matmuls — and with them
                # the ACT exp stream — never see a multi-us convoy of other
                # PE work nor lose their psum double-buffering, and the PE
                # activity stays dense enough to hold the HAM clock gate at
                # full rate.
                items = []
                if i >= 1:
                    pq0, pw, pp_ = units[i - 1]
                    items.append(
                        lambda pq0=pq0, pw=pw, pp_=pp_, j=i - 1: emit_norm_b(
                            pq0, pw, pp_, pend[j]
                        )
                    )
                for m in OSCHED.get(i, []):
                    items.append(lambda m=m: emit_final([m]))
                if i in (1, 3, 5):
                    qc = i // 2 + 1
                    items.append(lambda qc=qc: emit_qproj(qc, 0))
                    items.append(lambda qc=qc: emit_qproj(qc, 1))
                work = list(reversed(items))

                if i == 0:
                    hook = vhook
                else:
                    def hook(kt, work=work):
                        # start at slot 2 so the scores get a head start
                        if kt >= 2 and work:
                            work.pop()()

                pcA, pcB = emit_attn(q0, w, p, hook, lag=LAG0 if i == 0 else 2)
                while i > 0 and work:
                    work.pop()()
                # normalization part A immediately after this unit's ctx
                # stop: DVE+GPSIMD only, releases the psum banks fast
                pend[i] = emit_norm_a(q0, w, p, pcA, pcB)
            lq0, lw, lp = units[-1]
            emit_norm_b(lq0, lw, lp, pend[len(units) - 1])
            emit_final([15])

    nc.compile()
    return nc


def _const_rows():
    cst = np.zeros((3, 128), np.float32)
    cst[0, :] = 1.0
    cst[1, 0:64] = 1.0
    cst[2, 64:128] = 1.0
    return cst


def make_in_maps(query, key, value, mask, Wq, bq, Wk, bk, Wv, bv, Wo, bo):
    """Returns (in_maps, ktk). Key positions with mask=True are dropped."""
    query = np.asarray(query, np.float32)
    key = np.asarray(key, np.float32)
    value = np.asarray(value, np.float32)
    mask = np.asarray(mask)
    Wq = np.asarray(Wq, np.float32)
    Wk = np.asarray(Wk, np.float32)
    Wv = np.asarray(Wv, np.float32)
    Wo = np.asarray(Wo, np.float32)
    bq = np.asarray(bq, np.float32)
    bk = np.asarray(bk, np.float32)
    bv = np.asarray(bv, np.float32)

    keep = [np.flatnonzero(~mask[b, 0]) for b in range(B)]
    ktk = max(1, max((len(k) + 127) // 128 for k in keep))
    SKc = 128 * ktk
    ndt = _np_dt()
    if KF8:
        idt = mybir.dt.np(FP8)

        def _q8(a):
            return np.clip(a, -240.0, 240.0).astype(idt)
    else:
        idt = ndt
        _q8 = None

    def _prearrange(w):
        # [D, n] -> [128, KT_D, n] matching the SBUF tile layout
        n = w.shape[1]
        return np.ascontiguousarray(
            w.reshape(KT_D, 128, n).transpose(1, 0, 2)
        )

    in_maps = []
    for c in range(NCORES):
        b, g = c // G, c % G
        cs = slice(g * DG, (g + 1) * DG)
        idx = keep[b]
        nk = len(idx)
        xkc = np.zeros((D, SKc), np.float32)
        xvc = np.zeros((D, SKc), np.float32)
        xkc[:, :nk] = key[b].T[:, idx]
        xvc[:, :nk] = value[b].T[:, idx]
        mbias = np.full(SKc, MASK_NEG, np.float32)
        mbias[:nk] = 0.0

        wv_aug = np.zeros((D, VW), np.float32)
        wvb_row = np.zeros((1, VW), np.float32)
        for j in range(HPG):
            src = slice(g * DG + j * DK, g * DG + (j + 1) * DK)
            wv_aug[:, j * 65 : j * 65 + 64] = Wv[:, src]
            wvb_row[0, j * 65 : j * 65 + 64] = bv[src]
            wvb_row[0, j * 65 + 64] = 1.0

        # wo: [DG, D] -> [128, NT, D]
        wo_pre = np.ascontiguousarray(
            Wo[cs, :].reshape(NT, 128, D).transpose(1, 0, 2)
        )

        xq_c = np.ascontiguousarray(query[b].T)
        wq_c = _prearrange(Wq[:, cs])
        wk_c = _prearrange(Wk[:, cs])
        if KF8:
            xq_m, xk_m = _q8(xq_c), _q8(xkc)
            wq_m, wk_m = _q8(wq_c * WSC), _q8(wk_c * WSC)
        else:
            xq_m, xk_m = xq_c.astype(ndt), xkc.astype(ndt)
            wq_m, wk_m = wq_c.astype(ndt), wk_c.astype(ndt)

        in_maps.append(
            {
                "xq": xq_m,
                "xk": xk_m,
                "xv": xvc.astype(ndt),
                "wq": wq_m,
                "wk": wk_m,
                "wv": _prearrange(wv_aug).astype(ndt),
                "wvb": wvb_row.astype(ndt),
                "wo": wo_pre.astype(ndt),
                "bq": np.ascontiguousarray(bq[cs].reshape(NT, 128).T),
                "bk": np.ascontiguousarray(bk[cs].reshape(NT, 128).T),
                "mb": np.ascontiguousarray(mbias.reshape(ktk, 128).T),
                "cstc": _const_rows().astype(ndt),
            }
        )
    return in_maps, ktk


def combine_outputs(results, mask, bo):
    mask = np.asarray(mask)
    bo = np.asarray(bo, np.float32)
    out = np.zeros((B, S, D), np.float32)
    for c in range(NCORES):
        out[c // G] += np.asarray(results[c]["out"], np.float32)
    for b in range(B):
        if mask[b, 0].all():
            # reference: fully-masked rows produce zero context
            out[b] = 0.0
    out += bo[None, None, :]
    return out


_NC_CACHE = {}


def kernel(query, key, value, mask, Wq, bq, Wk, bk, Wv, bv, Wo, bo):
    from concourse.bass_utils import run_bass_kernel_spmd

    in_maps, ktk = make_in_maps(
        query, key, value, mask, Wq, bq, Wk, bk, Wv, bv, Wo, bo
    )
    nc = _NC_CACHE.get((KDT, KF8, ktk))
    if nc is None:
        nc = _NC_CACHE[(KDT, KF8, ktk)] = build_bass(ktk)
    res = run_bass_kernel_spmd(nc, in_maps, list(range(NCORES))).results
    return combine_outputs(res, mask, bo)
